# revision 11
# baseline (speedup 1.0000x reference)
"""Multi-head attention (S=2048, B=2, D=1024, H=16) on 8 TRN2 NeuronCores.

Sharding: batch*heads across cores — core c owns heads {2c, 2c+1} for both
batches (4 (head, batch) pairs per core, d_k=64 each -> a 128-row slice of
every projection). The output projection is row-parallel after an AllToAll
that redistributes per-head context to per-sequence-block context.

Per-core pipeline (single NEFF):
  1) QKV projections: Q^T/K^T/V^T [128hd, 4096i] = W_slice^T.T @ x^T  (bf16)
  2) V^T -> V_aug blocks [128j, 64d|1] via DMA-transpose (ones col -> softmax Z)
  3) Flash-style attention per (head, batch) pair in S^T orientation:
       S^T[j,i] tile = K^T_blk.T @ Q^T   (K=64 contraction)
       expS = ACT Exp(0.125 * S^T)  (psum -> sbuf bf16)
       ctxU^T[d|Z, i] += V_aug_blk.T @ expS  (K=128 contraction, M=65)
  4) normalize: ctx^T *= broadcast(1/Z)
  5) AllToAll: ctx^T columns for seq-block d -> core d
  6) O-proj: out[i,e] = ctx_full^T.T @ w_o^T + b_o  (fp32r), DMA out
"""

import numpy as np
import ml_dtypes

import concourse.bass as bass
import concourse.mybir as mybir
import concourse.tile as tile
from concourse import bacc
from concourse.bass_utils import run_bass_kernel_spmd

S = 2048
B = 2
D = 1024
H = 16
DK = 64
N_CORES = 8
SCALE = 1.0 / np.sqrt(DK)

F32 = mybir.dt.float32
F32R = mybir.dt.float32r
BF16 = mybir.dt.bfloat16

SB = S * B          # 4096 total rows (i = b*S + s)
ROWS_PER_CORE = SB // N_CORES   # 512 output rows per core (2 batches x 256)
SEQ_PER_CORE = S // N_CORES     # 256

_cached = {}


def build_program():
    if "nc" in _cached:
        return _cached["nc"]
    nc = bacc.Bacc("TRN2", target_bir_lowering=False, debug=False,
                   num_devices=N_CORES)

    # ---- DRAM I/O (per-core shards supplied by host) ----
    xT = {t: nc.dram_tensor(f"x{t}T", [D, SB], BF16, kind="ExternalInput")
          for t in "qkv"}
    wT = {t: nc.dram_tensor(f"w{t}T", [D, 128], BF16, kind="ExternalInput")
          for t in "qkv"}
    bvec = {t: nc.dram_tensor(f"b{t}", [128, 1], F32, kind="ExternalInput")
            for t in "qkv"}
    woT = nc.dram_tensor("woT", [D, D], F32R, kind="ExternalInput")
    bo_bc = nc.dram_tensor("bo_bc", [128, D], F32, kind="ExternalInput")
    out_d = nc.dram_tensor("out", [ROWS_PER_CORE, D], F32, kind="ExternalOutput")

    # internal DRAM bounce buffers for the collective
    a2a_in = nc.dram_tensor("a2a_in", [N_CORES * 128, ROWS_PER_CORE], F32R)
    a2a_out = nc.dram_tensor("a2a_out", [N_CORES * 128, ROWS_PER_CORE], F32R)

    with tile.TileContext(nc) as tc:
        _emit(nc, tc, xT, wT, bvec, woT, bo_bc, out_d, a2a_in, a2a_out)
    nc.compile()
    _cached["nc"] = nc
    return nc


def _emit(nc, tc, xT, wT, bvec, woT, bo_bc, out_d, a2a_in, a2a_out, dbg=None):
    from contextlib import ExitStack

    ICH = 1024          # i-chunk width for projections and attention
    NMM = 512           # max free dim per matmul into one PSUM bank
    JT = S // 128       # 16 j-tiles per pair

    with ExitStack() as top:
        const = top.enter_context(tc.tile_pool(name="const", bufs=1))
        # static SBUF tensors
        w_sb = const.tile([128, 3 * 8 * 128], BF16)     # w[qkv]T k-tiles
        bias_sb = const.tile([128, 3], F32)
        qT_sb = const.tile([128, SB], BF16)
        kT_sb = const.tile([128, SB], BF16)
        vT_sb = const.tile([128, SB], BF16)
        VA = 128        # V_aug block stride (xbar transpose needs aligned dest)
        vaug_sb = const.tile([128, 4 * JT * VA], BF16)  # V_aug blocks + ones col
        ctxT_sb = const.tile([128, SB], F32R)
        z_sb = const.tile([1, 4 * S], F32)
        zr_sb = const.tile([1, 4 * S], F32)
        bo_sb = const.tile([128, D], F32)
        osrc_sb = const.tile([128, N_CORES * ROWS_PER_CORE], F32R)

        for ti, t in enumerate("qkv"):
            for kt in range(8):
                nc.sync.dma_start(w_sb[:, (ti * 8 + kt) * 128:(ti * 8 + kt + 1) * 128],
                                  xw := wT[t].ap()[kt * 128:(kt + 1) * 128, :])
            nc.sync.dma_start(bias_sb[:, ti:ti + 1], bvec[t].ap())
        nc.sync.dma_start(bo_sb[:], bo_bc.ap())
        # ones column for every V_aug block (transposes later fill cols 0..63)
        nc.vector.memset(vaug_sb[:], 1.0)

        # ---------------- Phase 1: QKV projections ----------------
        # Q^T[hd, i] (and K,V): lhsT = w block [128f, 128hd], rhs = x^T [128f, ICH]
        proj_targets = {"q": qT_sb, "k": kT_sb, "v": vT_sb}
        with tc.tile_pool(name="xstream", bufs=6) as xpool, \
             tc.tile_pool(name="proj_psum", bufs=3, space="PSUM") as ppsum:
            for ich in range(SB // ICH):       # b0 chunks first -> early attn start
                for ti, t in enumerate("qkv"):
                    psum = ppsum.tile([128, ICH], F32)
                    for kt in range(8):
                        xtile = xpool.tile([128, ICH], BF16, tag="xs")
                        nc.sync.dma_start(
                            xtile[:],
                            xT[t].ap()[kt * 128:(kt + 1) * 128,
                                       ich * ICH:(ich + 1) * ICH])
                        for nn in range(ICH // NMM):
                            nc.tensor.matmul(
                                psum[:, nn * NMM:(nn + 1) * NMM],
                                w_sb[:, (ti * 8 + kt) * 128:(ti * 8 + kt + 1) * 128],
                                xtile[:, nn * NMM:(nn + 1) * NMM],
                                start=(kt == 0), stop=(kt == 7))
                    nc.vector.tensor_scalar_add(
                        proj_targets[t][:, ich * ICH:(ich + 1) * ICH],
                        psum[:], bias_sb[:, ti:ti + 1])

                # V_aug blocks for this chunk via DMA transpose (bf16 xbar):
                # chunk ich covers batch b = ich // 2, j in [off, off+1024)
                b = (ich * ICH) // S
                joff = (ich * ICH) % S
                for hh in range(2):
                    p = hh * 2 + b
                    for jt in range(joff // 128, (joff + ICH) // 128):
                        col = (p * JT + jt) * VA
                        nc.sync.dma_start_transpose(
                            vaug_sb[:, col:col + 64],
                            vT_sb[hh * 64:hh * 64 + 64,
                                  b * S + jt * 128:b * S + (jt + 1) * 128])

        # ---------------- Phase 2: attention ----------------
        with tc.tile_pool(name="spsum", bufs=2, space="PSUM") as spool, \
             tc.tile_pool(name="cpsum", bufs=2, space="PSUM") as cpool, \
             tc.tile_pool(name="zstage", bufs=2) as zstpool, \
             tc.tile_pool(name="expS", bufs=3) as epool:
            for b in range(B):
                for ch in range(S // ICH):
                    ioff = b * S + ch * ICH
                    cps = [cpool.tile([65, ICH], F32, tag="cp", name=f"cp{b}_{ch}_{i}")
                           for i in range(2)]
                    for jt in range(JT):
                        for hh in range(2):
                            p = hh * 2 + b
                            sps = spool.tile([128, ICH], F32, tag="sp")
                            for nn in range(ICH // NMM):
                                nc.tensor.matmul(
                                    sps[:, nn * NMM:(nn + 1) * NMM],
                                    kT_sb[hh * 64:hh * 64 + 64,
                                          b * S + jt * 128:b * S + (jt + 1) * 128],
                                    qT_sb[hh * 64:hh * 64 + 64,
                                          ioff + nn * NMM:ioff + (nn + 1) * NMM],
                                    start=True, stop=True)
                            es = epool.tile([128, ICH], BF16, tag="es")
                            nc.scalar.activation(
                                es[:], sps[:],
                                mybir.ActivationFunctionType.Exp, scale=float(SCALE))
                            col = (p * JT + jt) * VA
                            for nn in range(ICH // NMM):
                                nc.tensor.matmul(
                                    cps[hh][:, nn * NMM:(nn + 1) * NMM],
                                    vaug_sb[:, col:col + 65],
                                    es[:, nn * NMM:(nn + 1) * NMM],
                                    start=(jt == 0), stop=(jt == JT - 1))
                    for hh in range(2):
                        p = hh * 2 + b
                        nc.vector.tensor_copy(
                            ctxT_sb[hh * 64:hh * 64 + 64, ioff:ioff + ICH],
                            cps[hh][0:64, :])
                        # Z row: DVE can't cross partitions, DMA can't read
                        # PSUM -> stage row 64 in SBUF, then DMA to partition 0
                        zst = zstpool.tile([65, ICH], F32, tag="zst",
                                           name=f"zst{b}_{ch}_{hh}")
                        nc.vector.tensor_copy(zst[64:65, :], cps[hh][64:65, :])
                        nc.sync.dma_start(
                            z_sb[0:1, p * S + ch * ICH:p * S + (ch + 1) * ICH],
                            zst[64:65, :])

        # ---------------- Phase 3: normalize + AllToAll ----------------
        # Broadcast 1/Z along partitions via a K=1 matmul (ones ⊗ zr row):
        # zb[p, i] = 1 * zr[i] for all 128 partitions; then a partition-
        # aligned DVE multiply normalizes ctx^T per (head, batch) pair.
        nc.vector.reciprocal(zr_sb[:], z_sb[:])
        with tc.tile_pool(name="ones1", bufs=1) as onespool, \
             tc.tile_pool(name="zbc", bufs=2, space="PSUM") as zpool:
            ones1 = onespool.tile([1, 128], F32)
            nc.vector.memset(ones1[:], 1.0)
            for b in range(B):
                for hh in range(2):
                    p = hh * 2 + b
                    zb = zpool.tile([128, S], F32, tag="zbc", name=f"zb{b}{hh}")
                    for nn in range(S // NMM):
                        nc.tensor.matmul(
                            zb[:, nn * NMM:(nn + 1) * NMM], ones1[:],
                            zr_sb[0:1, p * S + nn * NMM:p * S + (nn + 1) * NMM],
                            start=True, stop=True)
                    nc.vector.tensor_mul(
                        ctxT_sb[hh * 64:hh * 64 + 64, b * S:(b + 1) * S],
                        ctxT_sb[hh * 64:hh * 64 + 64, b * S:(b + 1) * S],
                        zb[hh * 64:hh * 64 + 64, :])

        ctx_bs = ctxT_sb[:].rearrange("p (b s) -> p b s", b=B)
        for d in range(N_CORES):
            nc.sync.dma_start(
                a2a_in.ap()[d * 128:(d + 1) * 128, :]
                    .rearrange("p (b s) -> p b s", b=B),
                ctx_bs[:, :, d * SEQ_PER_CORE:(d + 1) * SEQ_PER_CORE])
        nc.gpsimd.collective_compute(
            "AllToAll", mybir.AluOpType.bypass,
            replica_groups=[list(range(N_CORES))],
            ins=[a2a_in.ap().opt()], outs=[a2a_out.ap().opt()])
        for s in range(N_CORES):
            nc.sync.dma_start(
                osrc_sb[:, s * ROWS_PER_CORE:(s + 1) * ROWS_PER_CORE],
                a2a_out.ap()[s * 128:(s + 1) * 128, :])

        if dbg is not None:
            nc.sync.dma_start(dbg["d_qT"].ap(), qT_sb[:])
            nc.sync.dma_start(dbg["d_kT"].ap(), kT_sb[:])
            nc.sync.dma_start(dbg["d_vT"].ap(), vT_sb[:])
            nc.sync.dma_start(dbg["d_vaug"].ap(), vaug_sb[:])
            nc.sync.dma_start(dbg["d_ctxT"].ap().bitcast(F32R), ctxT_sb[:])
            nc.sync.dma_start(dbg["d_z"].ap(), z_sb[:])
            nc.gpsimd.dma_start(dbg["d_a2a_out"].ap(), a2a_out.ap())

        # ---------------- Phase 4: output projection ----------------
        with tc.tile_pool(name="wo", bufs=3) as wopool, \
             tc.tile_pool(name="opsum", bufs=4, space="PSUM") as opool, \
             tc.tile_pool(name="oout", bufs=3) as outpool:
            for ce in range(D // NMM):
                psums = [opool.tile([128, NMM], F32, tag="op", name=f"op{ce}_{i}")
                         for i in range(4)]
                for s in range(N_CORES):
                    wo_t = wopool.tile([128, NMM], F32R, tag="wo")
                    nc.sync.dma_start(
                        wo_t[:], woT.ap()[s * 128:(s + 1) * 128,
                                          ce * NMM:(ce + 1) * NMM])
                    for it in range(4):
                        nc.tensor.matmul(
                            psums[it][:],
                            osrc_sb[:, s * ROWS_PER_CORE + it * 128:
                                    s * ROWS_PER_CORE + (it + 1) * 128],
                            wo_t[:], start=(s == 0), stop=(s == N_CORES - 1))
                for it in range(4):
                    ot = outpool.tile([128, NMM], F32, tag="ot")
                    nc.vector.tensor_add(ot[:], psums[it][:],
                                         bo_sb[:, ce * NMM:(ce + 1) * NMM])
                    nc.sync.dma_start(
                        out_d.ap()[it * 128:(it + 1) * 128,
                                   ce * NMM:(ce + 1) * NMM], ot[:])


def shard_inputs(inputs):
    q, k, v = inputs["query"], inputs["key"], inputs["value"]
    xt = {}
    for t, x in (("q", q), ("k", k), ("v", v)):
        # [S,B,D] -> x^T [D, B*S] with col = b*S + s, cast bf16
        xt[t] = np.ascontiguousarray(
            np.asarray(x, np.float32).transpose(2, 1, 0).reshape(D, SB)
        ).astype(ml_dtypes.bfloat16)
    woT = np.ascontiguousarray(np.asarray(inputs["w_o"], np.float32).T)
    bo_bc = np.ascontiguousarray(
        np.tile(np.asarray(inputs["b_o"], np.float32).reshape(1, D), (128, 1)))
    in_maps = []
    for c in range(N_CORES):
        m = {"woT": woT, "bo_bc": bo_bc}
        for t in "qkv":
            m[f"x{t}T"] = xt[t]
            w = np.asarray(inputs[f"w_{t}"], np.float32)
            bb = np.asarray(inputs[f"b_{t}"], np.float32)
            m[f"w{t}T"] = np.ascontiguousarray(
                w[c * 128:(c + 1) * 128, :].T).astype(ml_dtypes.bfloat16)
            m[f"b{t}"] = np.ascontiguousarray(
                bb[c * 128:(c + 1) * 128].reshape(128, 1))
        in_maps.append(m)
    return in_maps


def unshard(results):
    out = np.empty((S, B, D), np.float32)
    for c in range(N_CORES):
        o = results[c]["out"]          # [512, 1024], row r = b*256 + rr
        for b in range(B):
            out[c * SEQ_PER_CORE:(c + 1) * SEQ_PER_CORE, b, :] = \
                o[b * SEQ_PER_CORE:(b + 1) * SEQ_PER_CORE, :]
    return out


def run(inputs, trace=False, trace_cores=None):
    nc = build_program()
    in_maps = shard_inputs(inputs)
    res = run_bass_kernel_spmd(nc, in_maps, core_ids=list(range(N_CORES)),
                               trace=trace, trace_cores=trace_cores)
    return unshard(res.results), res


def kernel(**inputs):
    out, _ = run(inputs, trace=False)
    return out


# revision 14
# speedup vs baseline: 1.2043x; 1.2043x over previous
"""Multi-head attention (S=2048, B=2, D=1024, H=16) on 8 TRN2 NeuronCores.

Sharding: batch*heads across cores — core c owns heads {2c, 2c+1} for both
batches (4 (head, batch) pairs per core, d_k=64 each -> a 128-row slice of
every projection). The output projection is row-parallel after an AllToAll
that redistributes per-head context to per-sequence-block context.

Per-core pipeline (single NEFF), per batch b (pipelined so batch 0's
normalize/AllToAll/O-proj overlap batch 1's attention):
  1) QKV projections: Q^T/K^T/V^T [128hd, i] = W_slice^T.T @ x^T  (bf16)
  2) V^T -> V_aug blocks [128j, 64d|1] via DMA-transpose (ones col -> Z)
  3) Flash-style attention per (head, batch) pair in S^T orientation:
       S^T[j,i] tile = K^T_blk.T @ Q^T   (K=64 contraction)
       expS = ACT Exp(0.125 * S^T)  (psum -> sbuf bf16)
       ctxU^T[d|Z, i] += V_aug_blk.T @ expS  (K=128 contraction, M=65)
  4) Z -> [128,*] layout (DMA), reciprocal_approx_fast, back out to DRAM
     scratch, broadcast-read into [128, S] (step-0-source DMA), one DVE
     multiply normalizes + casts ctx^T to bf16
  5) AllToAll (bf16): ctx^T columns for seq-block d -> core d
  6) O-proj (bf16): out[i,e] = ctx_full^T.T @ w_o^T + b_o, DMA out
"""

import numpy as np
import ml_dtypes

import concourse.bass as bass
import concourse.mybir as mybir
import concourse.tile as tile
from concourse import bacc
from concourse.bass_utils import run_bass_kernel_spmd

S = 2048
B = 2
D = 1024
H = 16
DK = 64
N_CORES = 8
SCALE = 1.0 / np.sqrt(DK)

F32 = mybir.dt.float32
F32R = mybir.dt.float32r
BF16 = mybir.dt.bfloat16

SB = S * B          # 4096 total rows (i = b*S + s)
ROWS_PER_CORE = SB // N_CORES   # 512 output rows per core (2 batches x 256)
SEQ_PER_CORE = S // N_CORES     # 256

_cached = {}


def build_program():
    if "nc" in _cached:
        return _cached["nc"]
    nc = bacc.Bacc("TRN2", target_bir_lowering=False, debug=False,
                   num_devices=N_CORES)

    # ---- DRAM I/O (per-core shards supplied by host) ----
    xT = {t: nc.dram_tensor(f"x{t}T", [D, SB], BF16, kind="ExternalInput")
          for t in "qkv"}
    wT = {t: nc.dram_tensor(f"w{t}T", [D, 128], BF16, kind="ExternalInput")
          for t in "qkv"}
    bvec = {t: nc.dram_tensor(f"b{t}", [128, 1], F32, kind="ExternalInput")
            for t in "qkv"}
    woT = nc.dram_tensor("woT", [D, D], BF16, kind="ExternalInput")
    bo_bc = nc.dram_tensor("bo_bc", [128, D], F32, kind="ExternalInput")
    out_d = nc.dram_tensor("out", [ROWS_PER_CORE, D], F32, kind="ExternalOutput")

    # internal DRAM: per-batch collective bounce buffers + Z scratch
    a2a_in = [nc.dram_tensor(f"a2a_in{b}", [N_CORES * 128, SEQ_PER_CORE], BF16)
              for b in range(B)]
    a2a_out = [nc.dram_tensor(f"a2a_out{b}", [N_CORES * 128, SEQ_PER_CORE], BF16)
               for b in range(B)]
    zscr = [nc.dram_tensor(f"zscr{b}", [1, 2 * S], F32) for b in range(B)]

    with tile.TileContext(nc) as tc:
        _emit(nc, tc, xT, wT, bvec, woT, bo_bc, out_d, a2a_in, a2a_out, zscr)
    nc.compile()
    _cached["nc"] = nc
    return nc


def _emit(nc, tc, xT, wT, bvec, woT, bo_bc, out_d, a2a_in, a2a_out, zscr,
          dbg=None):
    from contextlib import ExitStack

    ICH = 1024          # i-chunk width for attention
    XCH = 2048          # x^T load width (one batch-half per DMA)
    NMM = 512           # max free dim per matmul into one PSUM bank
    JT = S // 128       # 16 j-tiles per pair
    VA = 128            # V_aug block stride (xbar transpose needs aligned dest)

    with ExitStack() as top:
        const = top.enter_context(tc.tile_pool(name="const", bufs=1))
        w_sb = const.tile([128, 3 * 8 * 128], BF16)     # w[qkv]T k-tiles
        bias_sb = const.tile([128, 3], F32)
        qT_sb = const.tile([128, SB], BF16)
        kT_sb = const.tile([128, SB], BF16)
        vT_sb = const.tile([128, SB], BF16)
        vaug_sb = const.tile([128, 4 * JT * VA], BF16)
        ctxU_sb = const.tile([128, SB], F32)            # unnormalized ctx^T
        ctxn_sb = const.tile([128, SB], BF16)           # normalized (bf16)
        bo_sb = const.tile([128, D], F32)
        osrc_sb = const.tile([128, B * N_CORES * SEQ_PER_CORE], BF16)

        for ti, t in enumerate("qkv"):
            for kt in range(8):
                nc.sync.dma_start(
                    w_sb[:, (ti * 8 + kt) * 128:(ti * 8 + kt + 1) * 128],
                    wT[t].ap()[kt * 128:(kt + 1) * 128, :])
            nc.sync.dma_start(bias_sb[:, ti:ti + 1], bvec[t].ap())
        nc.sync.dma_start(bo_sb[:], bo_bc.ap())
        nc.vector.memset(vaug_sb[:], 1.0)

        wo_sb = const.tile([128, 2 * N_CORES * NMM], BF16)   # preloaded w_o^T
        for ce in range(D // NMM):
            for s in range(N_CORES):
                nc.sync.dma_start(
                    wo_sb[:, (ce * N_CORES + s) * NMM:
                          (ce * N_CORES + s + 1) * NMM],
                    woT.ap()[s * 128:(s + 1) * 128, ce * NMM:(ce + 1) * NMM])

        xpool = top.enter_context(tc.tile_pool(name="xstream", bufs=5))
        zstpool = top.enter_context(tc.tile_pool(name="zstage", bufs=2))
        epool = top.enter_context(tc.tile_pool(name="expS", bufs=3))
        zbpool = top.enter_context(tc.tile_pool(name="zbc", bufs=2))
        outpool = top.enter_context(tc.tile_pool(name="oout", bufs=3))
        pools = {}

        proj_targets = {"q": qT_sb, "k": kT_sb, "v": vT_sb}

        def emit_proj(b):
            x0 = b * S
            for ti, t in enumerate("qkv"):
                psums = [pools["pp"].tile([128, ICH], F32, tag="pp",
                                    name=f"pp{b}_{t}_{i}") for i in range(2)]
                for kt in range(8):
                    xtile = xpool.tile([128, XCH], BF16, tag="xs",
                                       name=f"xs{b}_{t}_{kt}")
                    eng = nc.sync if (kt % 2 == 0) else nc.scalar
                    eng.dma_start(xtile[:],
                                  xT[t].ap()[kt * 128:(kt + 1) * 128,
                                             x0:x0 + XCH])
                    for ch in range(2):
                        for nn in range(2):
                            off = ch * ICH + nn * NMM
                            nc.tensor.matmul(
                                psums[ch][:, nn * NMM:(nn + 1) * NMM],
                                w_sb[:, (ti * 8 + kt) * 128:
                                     (ti * 8 + kt + 1) * 128],
                                xtile[:, off:off + NMM],
                                start=(kt == 0), stop=(kt == 7))
                for ch in range(2):
                    nc.vector.tensor_scalar_add(
                        proj_targets[t][:, x0 + ch * ICH:x0 + (ch + 1) * ICH],
                        psums[ch][:], bias_sb[:, ti:ti + 1])
            # V_aug blocks via bf16 xbar DMA-transpose (on the scalar HWDGE)
            for hh in range(2):
                p = hh * 2 + b
                for jt in range(JT):
                    col = (p * JT + jt) * VA
                    nc.scalar.dma_start_transpose(
                        vaug_sb[:, col:col + 64],
                        vT_sb[hh * 64:hh * 64 + 64,
                              x0 + jt * 128:x0 + (jt + 1) * 128])

        def emit_attention(b):
            for ch in range(S // ICH):
                ioff = b * S + ch * ICH
                cps = [pools["cp"].tile([65, ICH], F32, tag="cp",
                                  name=f"cp{b}_{ch}_{i}") for i in range(2)]
                for jt in range(JT):
                    for hh in range(2):
                        p = hh * 2 + b
                        sps = pools["sp"].tile([128, ICH], F32, tag="sp",
                                         name=f"sp{b}_{ch}_{jt}_{hh}")
                        for nn in range(2):
                            nc.tensor.matmul(
                                sps[:, nn * NMM:(nn + 1) * NMM],
                                kT_sb[hh * 64:hh * 64 + 64,
                                      b * S + jt * 128:b * S + (jt + 1) * 128],
                                qT_sb[hh * 64:hh * 64 + 64,
                                      ioff + nn * NMM:ioff + (nn + 1) * NMM],
                                start=True, stop=True)
                        es = epool.tile([128, ICH], BF16, tag="es",
                                        name=f"es{b}_{ch}_{jt}_{hh}")
                        nc.scalar.activation(
                            es[:], sps[:], mybir.ActivationFunctionType.Exp,
                            scale=float(SCALE))
                        col = (p * JT + jt) * VA
                        for nn in range(2):
                            nc.tensor.matmul(
                                cps[hh][:, nn * NMM:(nn + 1) * NMM],
                                vaug_sb[:, col:col + 65],
                                es[:, nn * NMM:(nn + 1) * NMM],
                                start=(jt == 0), stop=(jt == JT - 1))
                for hh in range(2):
                    q = b * 2 + hh      # batch-major Z block index
                    nc.vector.tensor_copy(
                        ctxU_sb[hh * 64:hh * 64 + 64, ioff:ioff + ICH],
                        cps[hh][0:64, :])
                    # Z row lives on psum partition 64: DVE-stage it in SBUF,
                    # then DMA into partition-major layout z2[pp, cc]
                    # holding i = cc*128 + pp.
                    zst = zstpool.tile([65, ICH], F32, tag="zst",
                                       name=f"zst{b}_{ch}_{hh}")
                    nc.vector.tensor_copy(zst[64:65, :], cps[hh][64:65, :])
                    nc.sync.dma_start(
                        zscr[b].ap()[0:1, hh * S + ch * ICH:
                                     hh * S + (ch + 1) * ICH],
                        zst[64:65, :])

        def emit_normalize_a2a(b):
            # Broadcast-read raw Z from DRAM (step-0 source AP) so rows
            # hh*64..hh*64+63 hold Z of pair (hh, b); then 1/Z via the fast
            # custom-DVE reciprocal and one multiply normalizes + casts bf16.
            zbc = zbpool.tile([128, S], F32, tag="zbc", name=f"zbc{b}")
            nc.sync.dma_start(
                zbc[:], bass.AP(zscr[b].ap().tensor, 0,
                                [[S, 2], [0, 64], [1, S]]))
            zbr = zbpool.tile([128, S], F32, tag="zbr", name=f"zbr{b}")
            nc.vector.reciprocal_approx_fast(zbr[:], zbc[:])
            nc.vector.tensor_mul(ctxn_sb[:, b * S:(b + 1) * S],
                                 ctxU_sb[:, b * S:(b + 1) * S], zbr[:])
            for d in range(N_CORES):
                nc.sync.dma_start(
                    a2a_in[b].ap()[d * 128:(d + 1) * 128, :],
                    ctxn_sb[:, b * S + d * SEQ_PER_CORE:
                            b * S + (d + 1) * SEQ_PER_CORE])
            nc.gpsimd.collective_compute(
                "AllToAll", mybir.AluOpType.bypass,
                replica_groups=[list(range(N_CORES))],
                ins=[a2a_in[b].ap().opt()], outs=[a2a_out[b].ap().opt()])
            for s in range(N_CORES):
                nc.sync.dma_start(
                    osrc_sb[:, (b * N_CORES + s) * SEQ_PER_CORE:
                            (b * N_CORES + s + 1) * SEQ_PER_CORE],
                    a2a_out[b].ap()[s * 128:(s + 1) * 128, :])

        def emit_oproj(b):
            for ce in range(D // NMM):
                psums = [pools["op"].tile([128, NMM], F32, tag="op",
                                    name=f"op{b}_{ce}_{i}") for i in range(2)]
                for s in range(N_CORES):
                    wo_t = wo_sb[:, (ce * N_CORES + s) * NMM:
                                 (ce * N_CORES + s + 1) * NMM]
                    for it in range(2):
                        nc.tensor.matmul(
                            psums[it][:],
                            osrc_sb[:, (b * N_CORES + s) * SEQ_PER_CORE +
                                    it * 128:
                                    (b * N_CORES + s) * SEQ_PER_CORE +
                                    (it + 1) * 128],
                            wo_t, start=(s == 0), stop=(s == N_CORES - 1))
                for it in range(2):
                    ot = outpool.tile([128, NMM], F32, tag="ot",
                                      name=f"ot{b}_{ce}_{it}")
                    nc.vector.tensor_add(ot[:], psums[it][:],
                                         bo_sb[:, ce * NMM:(ce + 1) * NMM])
                    nc.sync.dma_start(
                        out_d.ap()[b * SEQ_PER_CORE + it * 128:
                                   b * SEQ_PER_CORE + (it + 1) * 128,
                                   ce * NMM:(ce + 1) * NMM], ot[:])

        with tc.tile_pool(name="proj_psum", bufs=3, space="PSUM") as pp:
            pools["pp"] = pp
            emit_proj(0)
            emit_proj(1)
        with tc.tile_pool(name="spsum", bufs=2, space="PSUM") as sp, \
             tc.tile_pool(name="cpsum", bufs=2, space="PSUM") as cp:
            pools["sp"], pools["cp"] = sp, cp
            emit_attention(0)
            emit_normalize_a2a(0)
            emit_attention(1)
            emit_normalize_a2a(1)
        with tc.tile_pool(name="opsum", bufs=4, space="PSUM") as op:
            pools["op"] = op
            emit_oproj(0)
            emit_oproj(1)

        if dbg is not None:
            nc.sync.dma_start(dbg["d_qT"].ap(), qT_sb[:])
            nc.sync.dma_start(dbg["d_kT"].ap(), kT_sb[:])
            nc.sync.dma_start(dbg["d_vT"].ap(), vT_sb[:])
            nc.sync.dma_start(dbg["d_vaug"].ap(), vaug_sb[:])
            nc.sync.dma_start(dbg["d_ctxT"].ap(), ctxn_sb[:])


def shard_inputs(inputs):
    q, k, v = inputs["query"], inputs["key"], inputs["value"]
    xt = {}
    for t, x in (("q", q), ("k", k), ("v", v)):
        xt[t] = np.ascontiguousarray(
            np.asarray(x, np.float32).transpose(2, 1, 0).reshape(D, SB)
        ).astype(ml_dtypes.bfloat16)
    woT = np.ascontiguousarray(
        np.asarray(inputs["w_o"], np.float32).T).astype(ml_dtypes.bfloat16)
    bo_bc = np.ascontiguousarray(
        np.tile(np.asarray(inputs["b_o"], np.float32).reshape(1, D), (128, 1)))
    in_maps = []
    for c in range(N_CORES):
        m = {"woT": woT, "bo_bc": bo_bc}
        for t in "qkv":
            m[f"x{t}T"] = xt[t]
            w = np.asarray(inputs[f"w_{t}"], np.float32)
            bb = np.asarray(inputs[f"b_{t}"], np.float32)
            m[f"w{t}T"] = np.ascontiguousarray(
                w[c * 128:(c + 1) * 128, :].T).astype(ml_dtypes.bfloat16)
            m[f"b{t}"] = np.ascontiguousarray(
                bb[c * 128:(c + 1) * 128].reshape(128, 1))
        in_maps.append(m)
    return in_maps


def unshard(results):
    out = np.empty((S, B, D), np.float32)
    for c in range(N_CORES):
        o = results[c]["out"]          # [512, 1024], row r = b*256 + rr
        for b in range(B):
            out[c * SEQ_PER_CORE:(c + 1) * SEQ_PER_CORE, b, :] = \
                o[b * SEQ_PER_CORE:(b + 1) * SEQ_PER_CORE, :]
    return out


def run(inputs, trace=False, trace_cores=None):
    nc = build_program()
    in_maps = shard_inputs(inputs)
    res = run_bass_kernel_spmd(nc, in_maps, core_ids=list(range(N_CORES)),
                               trace=trace, trace_cores=trace_cores)
    return unshard(res.results), res


def kernel(**inputs):
    out, _ = run(inputs, trace=False)
    return out


# revision 15
# speedup vs baseline: 1.2353x; 1.0257x over previous
"""Multi-head attention (S=2048, B=2, D=1024, H=16) on 8 TRN2 NeuronCores.

Sharding: batch*heads across cores — core c owns heads {2c, 2c+1} for both
batches (4 (head, batch) pairs per core, d_k=64 each -> a 128-row slice of
every projection). The output projection is row-parallel after an AllToAll
that redistributes per-head context to per-sequence-block context.

Per-core pipeline (single NEFF), per batch b (pipelined so batch 0's
normalize/AllToAll/O-proj overlap batch 1's attention):
  1) QKV projections: Q^T/K^T/V^T [128hd, i] = W_slice^T.T @ x^T  (bf16)
  2) V^T -> V_aug blocks [128j, 64d|1] via DMA-transpose (ones col -> Z)
  3) Flash-style attention per (head, batch) pair in S^T orientation:
       S^T[j,i] tile = K^T_blk.T @ Q^T   (K=64 contraction)
       expS = ACT Exp(0.125 * S^T)  (psum -> sbuf bf16)
       ctxU^T[d|Z, i] += V_aug_blk.T @ expS  (K=128 contraction, M=65)
  4) Z -> [128,*] layout (DMA), reciprocal_approx_fast, back out to DRAM
     scratch, broadcast-read into [128, S] (step-0-source DMA), one DVE
     multiply normalizes + casts ctx^T to bf16
  5) AllToAll (bf16): ctx^T columns for seq-block d -> core d
  6) O-proj (bf16): out[i,e] = ctx_full^T.T @ w_o^T + b_o, DMA out
"""

import numpy as np
import ml_dtypes

import concourse.bass as bass
import concourse.mybir as mybir
import concourse.tile as tile
from concourse import bacc
from concourse.bass_utils import run_bass_kernel_spmd

S = 2048
B = 2
D = 1024
H = 16
DK = 64
N_CORES = 8
SCALE = 1.0 / np.sqrt(DK)

F32 = mybir.dt.float32
F32R = mybir.dt.float32r
BF16 = mybir.dt.bfloat16

SB = S * B          # 4096 total rows (i = b*S + s)
ROWS_PER_CORE = SB // N_CORES   # 512 output rows per core (2 batches x 256)
SEQ_PER_CORE = S // N_CORES     # 256

_cached = {}


def build_program():
    if "nc" in _cached:
        return _cached["nc"]
    nc = bacc.Bacc("TRN2", target_bir_lowering=False, debug=False,
                   num_devices=N_CORES)

    # ---- DRAM I/O (per-core shards supplied by host) ----
    xT = {t: nc.dram_tensor(f"x{t}T", [D, SB], BF16, kind="ExternalInput")
          for t in "qkv"}
    wT = {t: nc.dram_tensor(f"w{t}T", [D, 128], BF16, kind="ExternalInput")
          for t in "qkv"}
    bvec = {t: nc.dram_tensor(f"b{t}", [128, 1], F32, kind="ExternalInput")
            for t in "qkv"}
    woT = nc.dram_tensor("woT", [D, D], BF16, kind="ExternalInput")
    bo_bc = nc.dram_tensor("bo_bc", [128, D], F32, kind="ExternalInput")
    out_d = nc.dram_tensor("out", [ROWS_PER_CORE, D], F32, kind="ExternalOutput")

    # internal DRAM: per-batch collective bounce buffers + Z scratch
    a2a_in = [nc.dram_tensor(f"a2a_in{b}", [N_CORES * 128, SEQ_PER_CORE], BF16)
              for b in range(B)]
    a2a_out = [nc.dram_tensor(f"a2a_out{b}", [N_CORES * 128, SEQ_PER_CORE], BF16)
               for b in range(B)]
    zscr = [nc.dram_tensor(f"zscr{b}", [1, 2 * S], F32) for b in range(B)]

    with tile.TileContext(nc) as tc:
        _emit(nc, tc, xT, wT, bvec, woT, bo_bc, out_d, a2a_in, a2a_out, zscr)
    nc.compile()
    _cached["nc"] = nc
    return nc


def _emit(nc, tc, xT, wT, bvec, woT, bo_bc, out_d, a2a_in, a2a_out, zscr,
          dbg=None):
    from contextlib import ExitStack

    ICH = 1024          # i-chunk width for attention
    XCH = 2048          # x^T load width (one batch-half per DMA)
    NMM = 512           # max free dim per matmul into one PSUM bank
    JT = S // 128       # 16 j-tiles per pair
    VA = 128            # V_aug block stride (xbar transpose needs aligned dest)

    with ExitStack() as top:
        const = top.enter_context(tc.tile_pool(name="const", bufs=1))
        w_sb = const.tile([128, 3 * 8 * 128], BF16)     # w[qkv]T k-tiles
        bias_sb = const.tile([128, 3], F32)
        qT_sb = const.tile([128, SB], BF16)
        kT_sb = const.tile([128, SB], BF16)
        vT_sb = const.tile([128, SB], BF16)
        vaug_sb = const.tile([128, 4 * JT * VA], BF16)
        ctxU_sb = const.tile([128, SB], F32)            # unnormalized ctx^T
        ctxn_sb = const.tile([128, SB], BF16)           # normalized (bf16)
        bo_sb = const.tile([128, D], F32)
        osrc_sb = const.tile([128, B * N_CORES * SEQ_PER_CORE], BF16)

        for ti, t in enumerate("qkv"):
            for kt in range(8):
                nc.sync.dma_start(
                    w_sb[:, (ti * 8 + kt) * 128:(ti * 8 + kt + 1) * 128],
                    wT[t].ap()[kt * 128:(kt + 1) * 128, :])
            nc.sync.dma_start(bias_sb[:, ti:ti + 1], bvec[t].ap())
        nc.sync.dma_start(bo_sb[:], bo_bc.ap())
        nc.vector.memset(vaug_sb[:], 1.0)

        wo_sb = const.tile([128, 2 * N_CORES * NMM], BF16)   # preloaded w_o^T
        for ce in range(D // NMM):
            for s in range(N_CORES):
                nc.sync.dma_start(
                    wo_sb[:, (ce * N_CORES + s) * NMM:
                          (ce * N_CORES + s + 1) * NMM],
                    woT.ap()[s * 128:(s + 1) * 128, ce * NMM:(ce + 1) * NMM])

        xpool = top.enter_context(tc.tile_pool(name="xstream", bufs=4))
        zstpool = top.enter_context(tc.tile_pool(name="zstage", bufs=2))
        epool = top.enter_context(tc.tile_pool(name="expS", bufs=3))
        zbpool = top.enter_context(tc.tile_pool(name="zbc", bufs=2))
        outpool = top.enter_context(tc.tile_pool(name="oout", bufs=3))
        pools = {}

        proj_targets = {"q": qT_sb, "k": kT_sb, "v": vT_sb}

        def emit_proj():
            engines = {"v": nc.gpsimd, "k": nc.scalar, "q": nc.sync}
            for t in "vkq":            # V first so transposes overlap k/q proj
                ti = "qkv".index(t)
                psums = [pools["pp"].tile([128, ICH], F32, tag="pp",
                                          name=f"pp_{t}_{i}") for i in range(4)]
                for kt in range(8):
                    xtile = xpool.tile([128, SB], BF16, tag="xs",
                                       name=f"xs_{t}_{kt}")
                    engines[t].dma_start(xtile[:],
                                         xT[t].ap()[kt * 128:(kt + 1) * 128, :])
                    for ch in range(4):
                        for nn in range(2):
                            off = ch * ICH + nn * NMM
                            nc.tensor.matmul(
                                psums[ch][:, nn * NMM:(nn + 1) * NMM],
                                w_sb[:, (ti * 8 + kt) * 128:
                                     (ti * 8 + kt + 1) * 128],
                                xtile[:, off:off + NMM],
                                start=(kt == 0), stop=(kt == 7))
                for ch in range(4):
                    nc.vector.tensor_scalar_add(
                        proj_targets[t][:, ch * ICH:(ch + 1) * ICH],
                        psums[ch][:], bias_sb[:, ti:ti + 1])
                if t == "v":
                    # V_aug blocks via bf16 xbar DMA-transpose (sync HWDGE)
                    for b in range(B):
                        for hh in range(2):
                            p = hh * 2 + b
                            for jt in range(JT):
                                col = (p * JT + jt) * VA
                                nc.sync.dma_start_transpose(
                                    vaug_sb[:, col:col + 64],
                                    vT_sb[hh * 64:hh * 64 + 64,
                                          b * S + jt * 128:
                                          b * S + (jt + 1) * 128])

        def emit_attention(b):
            for ch in range(S // ICH):
                ioff = b * S + ch * ICH
                cps = [pools["cp"].tile([65, ICH], F32, tag="cp",
                                  name=f"cp{b}_{ch}_{i}") for i in range(2)]
                for jt in range(JT):
                    for hh in range(2):
                        p = hh * 2 + b
                        sps = pools["sp"].tile([128, ICH], F32, tag="sp",
                                         name=f"sp{b}_{ch}_{jt}_{hh}")
                        for nn in range(2):
                            nc.tensor.matmul(
                                sps[:, nn * NMM:(nn + 1) * NMM],
                                kT_sb[hh * 64:hh * 64 + 64,
                                      b * S + jt * 128:b * S + (jt + 1) * 128],
                                qT_sb[hh * 64:hh * 64 + 64,
                                      ioff + nn * NMM:ioff + (nn + 1) * NMM],
                                start=True, stop=True)
                        es = epool.tile([128, ICH], BF16, tag="es",
                                        name=f"es{b}_{ch}_{jt}_{hh}")
                        nc.scalar.activation(
                            es[:], sps[:], mybir.ActivationFunctionType.Exp,
                            scale=float(SCALE))
                        col = (p * JT + jt) * VA
                        for nn in range(2):
                            nc.tensor.matmul(
                                cps[hh][:, nn * NMM:(nn + 1) * NMM],
                                vaug_sb[:, col:col + 65],
                                es[:, nn * NMM:(nn + 1) * NMM],
                                start=(jt == 0), stop=(jt == JT - 1))
                for hh in range(2):
                    q = b * 2 + hh      # batch-major Z block index
                    nc.vector.tensor_copy(
                        ctxU_sb[hh * 64:hh * 64 + 64, ioff:ioff + ICH],
                        cps[hh][0:64, :])
                    # Z row lives on psum partition 64: DVE-stage it in SBUF,
                    # then DMA into partition-major layout z2[pp, cc]
                    # holding i = cc*128 + pp.
                    zst = zstpool.tile([65, ICH], F32, tag="zst",
                                       name=f"zst{b}_{ch}_{hh}")
                    nc.vector.tensor_copy(zst[64:65, :], cps[hh][64:65, :])
                    nc.sync.dma_start(
                        zscr[b].ap()[0:1, hh * S + ch * ICH:
                                     hh * S + (ch + 1) * ICH],
                        zst[64:65, :])

        def emit_normalize_a2a(b):
            # Broadcast-read raw Z from DRAM (step-0 source AP) so rows
            # hh*64..hh*64+63 hold Z of pair (hh, b); then 1/Z via the fast
            # custom-DVE reciprocal and one multiply normalizes + casts bf16.
            zbc = zbpool.tile([128, S], F32, tag="zbc", name=f"zbc{b}")
            for cq in range(4):
                nc.sync.dma_start(
                    zbc[:, cq * (S // 4):(cq + 1) * (S // 4)],
                    bass.AP(zscr[b].ap().tensor, cq * (S // 4),
                            [[S, 2], [0, 64], [1, S // 4]]))
            zbr = zbpool.tile([128, S], F32, tag="zbr", name=f"zbr{b}")
            nc.vector.reciprocal_approx_fast(zbr[:], zbc[:])
            nc.vector.tensor_mul(ctxn_sb[:, b * S:(b + 1) * S],
                                 ctxU_sb[:, b * S:(b + 1) * S], zbr[:])
            for d in range(N_CORES):
                nc.sync.dma_start(
                    a2a_in[b].ap()[d * 128:(d + 1) * 128, :],
                    ctxn_sb[:, b * S + d * SEQ_PER_CORE:
                            b * S + (d + 1) * SEQ_PER_CORE])
            nc.gpsimd.collective_compute(
                "AllToAll", mybir.AluOpType.bypass,
                replica_groups=[list(range(N_CORES))],
                ins=[a2a_in[b].ap().opt()], outs=[a2a_out[b].ap().opt()])
            for s in range(N_CORES):
                nc.sync.dma_start(
                    osrc_sb[:, (b * N_CORES + s) * SEQ_PER_CORE:
                            (b * N_CORES + s + 1) * SEQ_PER_CORE],
                    a2a_out[b].ap()[s * 128:(s + 1) * 128, :])

        def emit_oproj(b):
            for ce in range(D // NMM):
                psums = [pools["op"].tile([128, NMM], F32, tag="op",
                                    name=f"op{b}_{ce}_{i}") for i in range(2)]
                for s in range(N_CORES):
                    wo_t = wo_sb[:, (ce * N_CORES + s) * NMM:
                                 (ce * N_CORES + s + 1) * NMM]
                    for it in range(2):
                        nc.tensor.matmul(
                            psums[it][:],
                            osrc_sb[:, (b * N_CORES + s) * SEQ_PER_CORE +
                                    it * 128:
                                    (b * N_CORES + s) * SEQ_PER_CORE +
                                    (it + 1) * 128],
                            wo_t, start=(s == 0), stop=(s == N_CORES - 1))
                for it in range(2):
                    ot = outpool.tile([128, NMM], F32, tag="ot",
                                      name=f"ot{b}_{ce}_{it}")
                    nc.vector.tensor_add(ot[:], psums[it][:],
                                         bo_sb[:, ce * NMM:(ce + 1) * NMM])
                    nc.sync.dma_start(
                        out_d.ap()[b * SEQ_PER_CORE + it * 128:
                                   b * SEQ_PER_CORE + (it + 1) * 128,
                                   ce * NMM:(ce + 1) * NMM], ot[:])

        with tc.tile_pool(name="proj_psum", bufs=4, space="PSUM") as pp:
            pools["pp"] = pp
            emit_proj()
        with tc.tile_pool(name="spsum", bufs=2, space="PSUM") as sp, \
             tc.tile_pool(name="cpsum", bufs=2, space="PSUM") as cp:
            pools["sp"], pools["cp"] = sp, cp
            emit_attention(0)
            emit_normalize_a2a(0)
            emit_attention(1)
            emit_normalize_a2a(1)
        with tc.tile_pool(name="opsum", bufs=4, space="PSUM") as op:
            pools["op"] = op
            emit_oproj(0)
            emit_oproj(1)

        if dbg is not None:
            nc.sync.dma_start(dbg["d_qT"].ap(), qT_sb[:])
            nc.sync.dma_start(dbg["d_kT"].ap(), kT_sb[:])
            nc.sync.dma_start(dbg["d_vT"].ap(), vT_sb[:])
            nc.sync.dma_start(dbg["d_vaug"].ap(), vaug_sb[:])
            nc.sync.dma_start(dbg["d_ctxT"].ap(), ctxn_sb[:])


def shard_inputs(inputs):
    q, k, v = inputs["query"], inputs["key"], inputs["value"]
    xt = {}
    for t, x in (("q", q), ("k", k), ("v", v)):
        xt[t] = np.ascontiguousarray(
            np.asarray(x, np.float32).transpose(2, 1, 0).reshape(D, SB)
        ).astype(ml_dtypes.bfloat16)
    woT = np.ascontiguousarray(
        np.asarray(inputs["w_o"], np.float32).T).astype(ml_dtypes.bfloat16)
    bo_bc = np.ascontiguousarray(
        np.tile(np.asarray(inputs["b_o"], np.float32).reshape(1, D), (128, 1)))
    in_maps = []
    for c in range(N_CORES):
        m = {"woT": woT, "bo_bc": bo_bc}
        for t in "qkv":
            m[f"x{t}T"] = xt[t]
            w = np.asarray(inputs[f"w_{t}"], np.float32)
            bb = np.asarray(inputs[f"b_{t}"], np.float32)
            m[f"w{t}T"] = np.ascontiguousarray(
                w[c * 128:(c + 1) * 128, :].T).astype(ml_dtypes.bfloat16)
            m[f"b{t}"] = np.ascontiguousarray(
                bb[c * 128:(c + 1) * 128].reshape(128, 1))
        in_maps.append(m)
    return in_maps


def unshard(results):
    out = np.empty((S, B, D), np.float32)
    for c in range(N_CORES):
        o = results[c]["out"]          # [512, 1024], row r = b*256 + rr
        for b in range(B):
            out[c * SEQ_PER_CORE:(c + 1) * SEQ_PER_CORE, b, :] = \
                o[b * SEQ_PER_CORE:(b + 1) * SEQ_PER_CORE, :]
    return out


def run(inputs, trace=False, trace_cores=None):
    nc = build_program()
    in_maps = shard_inputs(inputs)
    res = run_bass_kernel_spmd(nc, in_maps, core_ids=list(range(N_CORES)),
                               trace=trace, trace_cores=trace_cores)
    return unshard(res.results), res


def kernel(**inputs):
    out, _ = run(inputs, trace=False)
    return out


# revision 16
# speedup vs baseline: 1.2898x; 1.0441x over previous
"""Multi-head attention (S=2048, B=2, D=1024, H=16) on 8 TRN2 NeuronCores.

Sharding: batch*heads across cores — core c owns heads {2c, 2c+1} for both
batches (4 (head, batch) pairs per core, d_k=64 each -> a 128-row slice of
every projection). The output projection is row-parallel after an AllToAll
that redistributes per-head context to per-sequence-block context.

Per-core pipeline (single NEFF), per batch b (pipelined so batch 0's
normalize/AllToAll/O-proj overlap batch 1's attention):
  1) QKV projections: Q^T/K^T/V^T [128hd, i] = W_slice^T.T @ x^T  (bf16)
  2) V^T -> V_aug blocks [128j, 64d|1] via DMA-transpose (ones col -> Z)
  3) Flash-style attention per (head, batch) pair in S^T orientation:
       S^T[j,i] tile = K^T_blk.T @ Q^T   (K=64 contraction)
       expS = ACT Exp(0.125 * S^T)  (psum -> sbuf bf16)
       ctxU^T[d|Z, i] += V_aug_blk.T @ expS  (K=128 contraction, M=65)
  4) Z -> [128,*] layout (DMA), reciprocal_approx_fast, back out to DRAM
     scratch, broadcast-read into [128, S] (step-0-source DMA), one DVE
     multiply normalizes + casts ctx^T to bf16
  5) AllToAll (bf16): ctx^T columns for seq-block d -> core d
  6) O-proj (bf16): out[i,e] = ctx_full^T.T @ w_o^T + b_o, DMA out
"""

import numpy as np
import ml_dtypes

import concourse.bass as bass
import concourse.mybir as mybir
import concourse.tile as tile
from concourse import bacc
from concourse.bass_utils import run_bass_kernel_spmd

S = 2048
B = 2
D = 1024
H = 16
DK = 64
N_CORES = 8
SCALE = 1.0 / np.sqrt(DK)

F32 = mybir.dt.float32
F32R = mybir.dt.float32r
BF16 = mybir.dt.bfloat16

SB = S * B          # 4096 total rows (i = b*S + s)
ROWS_PER_CORE = SB // N_CORES   # 512 output rows per core (2 batches x 256)
SEQ_PER_CORE = S // N_CORES     # 256

_cached = {}


def build_program():
    if "nc" in _cached:
        return _cached["nc"]
    nc = bacc.Bacc("TRN2", target_bir_lowering=False, debug=False,
                   num_devices=N_CORES)

    # ---- DRAM I/O (per-core shards supplied by host) ----
    xT = {t: nc.dram_tensor(f"x{t}T", [D, SB], BF16, kind="ExternalInput")
          for t in "qkv"}
    wT = {t: nc.dram_tensor(f"w{t}T", [D, 128], BF16, kind="ExternalInput")
          for t in "qkv"}
    bvec = {t: nc.dram_tensor(f"b{t}", [128, 1], F32, kind="ExternalInput")
            for t in "qkv"}
    woT = nc.dram_tensor("woT", [D, D], BF16, kind="ExternalInput")
    bo_bc = nc.dram_tensor("bo_bc", [128, D], F32, kind="ExternalInput")
    out_d = nc.dram_tensor("out", [ROWS_PER_CORE, D], F32, kind="ExternalOutput")

    # internal DRAM: per-batch collective bounce buffers + Z scratch
    a2a_in = [nc.dram_tensor(f"a2a_in{b}", [N_CORES * 128, SEQ_PER_CORE], BF16)
              for b in range(B)]
    a2a_out = [nc.dram_tensor(f"a2a_out{b}", [N_CORES * 128, SEQ_PER_CORE], BF16)
               for b in range(B)]
    zscr = [nc.dram_tensor(f"zscr{b}", [1, 2 * S], F32) for b in range(B)]

    with tile.TileContext(nc) as tc:
        _emit(nc, tc, xT, wT, bvec, woT, bo_bc, out_d, a2a_in, a2a_out, zscr)
    nc.compile()
    _cached["nc"] = nc
    return nc


def _emit(nc, tc, xT, wT, bvec, woT, bo_bc, out_d, a2a_in, a2a_out, zscr,
          dbg=None):
    from contextlib import ExitStack

    ICH = 1024          # i-chunk width for attention
    XCH = 2048          # x^T load width (one batch-half per DMA)
    NMM = 512           # max free dim per matmul into one PSUM bank
    JT = S // 128       # 16 j-tiles per pair
    VA = 128            # V_aug block stride (xbar transpose needs aligned dest)

    with ExitStack() as top:
        const = top.enter_context(tc.tile_pool(name="const", bufs=1))
        w_sb = const.tile([128, 3 * 8 * 128], BF16)     # w[qkv]T k-tiles
        bias_sb = const.tile([128, 3], F32)
        qT_sb = const.tile([128, SB], BF16)
        kT_sb = const.tile([128, SB], BF16)
        vT_sb = const.tile([128, SB], BF16)
        vaug_sb = const.tile([128, 4 * JT * VA], BF16)
        ctxU_sb = const.tile([128, SB], F32)            # unnormalized ctx^T
        ctxn_sb = const.tile([128, SB], BF16)           # normalized (bf16)
        bo_sb = const.tile([128, D], F32)
        osrc_sb = const.tile([128, B * N_CORES * SEQ_PER_CORE], BF16)

        for ti, t in enumerate("qkv"):
            for kt in range(8):
                nc.sync.dma_start(
                    w_sb[:, (ti * 8 + kt) * 128:(ti * 8 + kt + 1) * 128],
                    wT[t].ap()[kt * 128:(kt + 1) * 128, :])
            nc.sync.dma_start(bias_sb[:, ti:ti + 1], bvec[t].ap())
        nc.sync.dma_start(bo_sb[:], bo_bc.ap())
        nc.vector.memset(vaug_sb[:], 1.0)

        wo_sb = const.tile([128, 2 * N_CORES * NMM], BF16)   # preloaded w_o^T
        for ce in range(D // NMM):
            for s in range(N_CORES):
                nc.sync.dma_start(
                    wo_sb[:, (ce * N_CORES + s) * NMM:
                          (ce * N_CORES + s + 1) * NMM],
                    woT.ap()[s * 128:(s + 1) * 128, ce * NMM:(ce + 1) * NMM])

        xpool = top.enter_context(tc.tile_pool(name="xstream", bufs=4))
        zstpool = top.enter_context(tc.tile_pool(name="zstage", bufs=2))
        epool = top.enter_context(tc.tile_pool(name="expS", bufs=3))
        zbpool = top.enter_context(tc.tile_pool(name="zbc", bufs=2))
        outpool = top.enter_context(tc.tile_pool(name="oout", bufs=3))
        pools = {}

        proj_targets = {"q": qT_sb, "k": kT_sb, "v": vT_sb}

        def emit_proj():
            engines = {"v": nc.gpsimd, "k": nc.scalar, "q": nc.sync}
            for t in "vkq":            # V first so transposes overlap k/q proj
                ti = "qkv".index(t)
                psums = [pools["pp"].tile([128, ICH], F32, tag="pp",
                                          name=f"pp_{t}_{i}") for i in range(4)]
                for kt in range(8):
                    xtile = xpool.tile([128, SB], BF16, tag="xs",
                                       name=f"xs_{t}_{kt}")
                    engines[t].dma_start(xtile[:],
                                         xT[t].ap()[kt * 128:(kt + 1) * 128, :])
                    for ch in range(4):
                        for nn in range(2):
                            off = ch * ICH + nn * NMM
                            nc.tensor.matmul(
                                psums[ch][:, nn * NMM:(nn + 1) * NMM],
                                w_sb[:, (ti * 8 + kt) * 128:
                                     (ti * 8 + kt + 1) * 128],
                                xtile[:, off:off + NMM],
                                start=(kt == 0), stop=(kt == 7))
                for ch in range(4):
                    nc.vector.tensor_scalar_add(
                        proj_targets[t][:, ch * ICH:(ch + 1) * ICH],
                        psums[ch][:], bias_sb[:, ti:ti + 1])
        def emit_transposes():
            # V_aug blocks via bf16 xbar DMA-transpose. Emitted after all
            # x-loads so they sit behind them in the sync engine's program
            # order; attention consumes blocks in jt order while these
            # stream at ~1.2us each.
            for b in range(B):
                for jt in range(JT):
                    for hh in range(2):
                        p = hh * 2 + b
                        col = (p * JT + jt) * VA
                        nc.sync.dma_start_transpose(
                            vaug_sb[:, col:col + 64],
                            vT_sb[hh * 64:hh * 64 + 64,
                                  b * S + jt * 128:b * S + (jt + 1) * 128])

        def emit_attention(b):
            for ch in range(S // ICH):
                ioff = b * S + ch * ICH
                cps = [pools["cp"].tile([65, ICH], F32, tag="cp",
                                  name=f"cp{b}_{ch}_{i}") for i in range(2)]
                for jt in range(JT):
                    for hh in range(2):
                        p = hh * 2 + b
                        sps = pools["sp"].tile([128, ICH], F32, tag="sp",
                                         name=f"sp{b}_{ch}_{jt}_{hh}")
                        for nn in range(2):
                            nc.tensor.matmul(
                                sps[:, nn * NMM:(nn + 1) * NMM],
                                kT_sb[hh * 64:hh * 64 + 64,
                                      b * S + jt * 128:b * S + (jt + 1) * 128],
                                qT_sb[hh * 64:hh * 64 + 64,
                                      ioff + nn * NMM:ioff + (nn + 1) * NMM],
                                start=True, stop=True)
                        es = epool.tile([128, ICH], BF16, tag="es",
                                        name=f"es{b}_{ch}_{jt}_{hh}")
                        nc.scalar.activation(
                            es[:], sps[:], mybir.ActivationFunctionType.Exp,
                            scale=float(SCALE))
                        col = (p * JT + jt) * VA
                        for nn in range(2):
                            nc.tensor.matmul(
                                cps[hh][:, nn * NMM:(nn + 1) * NMM],
                                vaug_sb[:, col:col + 65],
                                es[:, nn * NMM:(nn + 1) * NMM],
                                start=(jt == 0), stop=(jt == JT - 1))
                for hh in range(2):
                    q = b * 2 + hh      # batch-major Z block index
                    nc.vector.tensor_copy(
                        ctxU_sb[hh * 64:hh * 64 + 64, ioff:ioff + ICH],
                        cps[hh][0:64, :])
                    # Z row lives on psum partition 64: DVE-stage it in SBUF,
                    # then DMA into partition-major layout z2[pp, cc]
                    # holding i = cc*128 + pp.
                    zst = zstpool.tile([65, ICH], F32, tag="zst",
                                       name=f"zst{b}_{ch}_{hh}")
                    nc.vector.tensor_copy(zst[64:65, :], cps[hh][64:65, :])
                    nc.sync.dma_start(
                        zscr[b].ap()[0:1, hh * S + ch * ICH:
                                     hh * S + (ch + 1) * ICH],
                        zst[64:65, :])

        def emit_normalize_a2a(b):
            # Broadcast-read raw Z from DRAM (step-0 source AP) so rows
            # hh*64..hh*64+63 hold Z of pair (hh, b); then 1/Z via the fast
            # custom-DVE reciprocal and one multiply normalizes + casts bf16.
            zbc = zbpool.tile([128, S], F32, tag="zbc", name=f"zbc{b}")
            for cq in range(4):
                nc.sync.dma_start(
                    zbc[:, cq * (S // 4):(cq + 1) * (S // 4)],
                    bass.AP(zscr[b].ap().tensor, cq * (S // 4),
                            [[S, 2], [0, 64], [1, S // 4]]))
            zbr = zbpool.tile([128, S], F32, tag="zbr", name=f"zbr{b}")
            nc.vector.reciprocal_approx_fast(zbr[:], zbc[:])
            nc.vector.tensor_mul(ctxn_sb[:, b * S:(b + 1) * S],
                                 ctxU_sb[:, b * S:(b + 1) * S], zbr[:])
            for d in range(N_CORES):
                nc.sync.dma_start(
                    a2a_in[b].ap()[d * 128:(d + 1) * 128, :],
                    ctxn_sb[:, b * S + d * SEQ_PER_CORE:
                            b * S + (d + 1) * SEQ_PER_CORE])
            nc.gpsimd.collective_compute(
                "AllToAll", mybir.AluOpType.bypass,
                replica_groups=[list(range(N_CORES))],
                ins=[a2a_in[b].ap().opt()], outs=[a2a_out[b].ap().opt()])
            for s in range(N_CORES):
                nc.sync.dma_start(
                    osrc_sb[:, (b * N_CORES + s) * SEQ_PER_CORE:
                            (b * N_CORES + s + 1) * SEQ_PER_CORE],
                    a2a_out[b].ap()[s * 128:(s + 1) * 128, :])

        def emit_oproj(b):
            for ce in range(D // NMM):
                psums = [pools["op"].tile([128, NMM], F32, tag="op",
                                    name=f"op{b}_{ce}_{i}") for i in range(2)]
                for s in range(N_CORES):
                    wo_t = wo_sb[:, (ce * N_CORES + s) * NMM:
                                 (ce * N_CORES + s + 1) * NMM]
                    for it in range(2):
                        nc.tensor.matmul(
                            psums[it][:],
                            osrc_sb[:, (b * N_CORES + s) * SEQ_PER_CORE +
                                    it * 128:
                                    (b * N_CORES + s) * SEQ_PER_CORE +
                                    (it + 1) * 128],
                            wo_t, start=(s == 0), stop=(s == N_CORES - 1))
                for it in range(2):
                    ot = outpool.tile([128, NMM], F32, tag="ot",
                                      name=f"ot{b}_{ce}_{it}")
                    nc.vector.tensor_add(ot[:], psums[it][:],
                                         bo_sb[:, ce * NMM:(ce + 1) * NMM])
                    nc.sync.dma_start(
                        out_d.ap()[b * SEQ_PER_CORE + it * 128:
                                   b * SEQ_PER_CORE + (it + 1) * 128,
                                   ce * NMM:(ce + 1) * NMM], ot[:])

        with tc.tile_pool(name="proj_psum", bufs=4, space="PSUM") as pp:
            pools["pp"] = pp
            emit_proj()
            emit_transposes()
        with tc.tile_pool(name="spsum", bufs=2, space="PSUM") as sp, \
             tc.tile_pool(name="cpsum", bufs=2, space="PSUM") as cp:
            pools["sp"], pools["cp"] = sp, cp
            emit_attention(0)
            emit_normalize_a2a(0)
            emit_attention(1)
            emit_normalize_a2a(1)
        with tc.tile_pool(name="opsum", bufs=4, space="PSUM") as op:
            pools["op"] = op
            emit_oproj(0)
            emit_oproj(1)

        if dbg is not None:
            nc.sync.dma_start(dbg["d_qT"].ap(), qT_sb[:])
            nc.sync.dma_start(dbg["d_kT"].ap(), kT_sb[:])
            nc.sync.dma_start(dbg["d_vT"].ap(), vT_sb[:])
            nc.sync.dma_start(dbg["d_vaug"].ap(), vaug_sb[:])
            nc.sync.dma_start(dbg["d_ctxT"].ap(), ctxn_sb[:])


def shard_inputs(inputs):
    q, k, v = inputs["query"], inputs["key"], inputs["value"]
    xt = {}
    for t, x in (("q", q), ("k", k), ("v", v)):
        xt[t] = np.ascontiguousarray(
            np.asarray(x, np.float32).transpose(2, 1, 0).reshape(D, SB)
        ).astype(ml_dtypes.bfloat16)
    woT = np.ascontiguousarray(
        np.asarray(inputs["w_o"], np.float32).T).astype(ml_dtypes.bfloat16)
    bo_bc = np.ascontiguousarray(
        np.tile(np.asarray(inputs["b_o"], np.float32).reshape(1, D), (128, 1)))
    in_maps = []
    for c in range(N_CORES):
        m = {"woT": woT, "bo_bc": bo_bc}
        for t in "qkv":
            m[f"x{t}T"] = xt[t]
            w = np.asarray(inputs[f"w_{t}"], np.float32)
            bb = np.asarray(inputs[f"b_{t}"], np.float32)
            m[f"w{t}T"] = np.ascontiguousarray(
                w[c * 128:(c + 1) * 128, :].T).astype(ml_dtypes.bfloat16)
            m[f"b{t}"] = np.ascontiguousarray(
                bb[c * 128:(c + 1) * 128].reshape(128, 1))
        in_maps.append(m)
    return in_maps


def unshard(results):
    out = np.empty((S, B, D), np.float32)
    for c in range(N_CORES):
        o = results[c]["out"]          # [512, 1024], row r = b*256 + rr
        for b in range(B):
            out[c * SEQ_PER_CORE:(c + 1) * SEQ_PER_CORE, b, :] = \
                o[b * SEQ_PER_CORE:(b + 1) * SEQ_PER_CORE, :]
    return out


def run(inputs, trace=False, trace_cores=None):
    nc = build_program()
    in_maps = shard_inputs(inputs)
    res = run_bass_kernel_spmd(nc, in_maps, core_ids=list(range(N_CORES)),
                               trace=trace, trace_cores=trace_cores)
    return unshard(res.results), res


def kernel(**inputs):
    out, _ = run(inputs, trace=False)
    return out
